# revision 1
# baseline (speedup 1.0000x reference)
"""BinaryLinear forward for Trainium2, 8-core SPMD.

Reference computation (per problem):
    scale = mean(|W|)                    # scalar over full W
    out   = x @ (sign(W) * scale).T      # x [8, 2048, 4096], W [4096, 4096]

Sharding: data-parallel over the leading batch dim (8 batches -> 8 cores).
Each core computes out_b = x_b @ (sign(W) * scale).T with x_b [2048, 4096]
and the full W replicated. The scalar `scale` needs all of W; each core
reduces |W| over its own 1/8 row-slice (passed as an extra sharded input so
the access pattern stays static) and an AllReduce combines the partials.

Device algorithm per core:
  - x tiles are cast fp32->bf16 during DMA (SWDGE cast), then transposed
    SBUF->SBUF with the DMA xbar into XT k-major tiles (lhsT layout).
  - W row-tiles are cast-DMA'd to bf16, xbar-transposed into WT chunks
    (rhs layout), then sign() is applied in place on the ACT engine.
    sign(W)*scale is folded as: matmul against sign(W) (exact +-1 in bf16),
    scale applied on the psum->SBUF eviction (ACT copy with per-partition
    scale). bf16 is near-lossless here: sign is exact, only x rounds.
  - PE does K=128-deep accumulating matmuls: psum[m=128, n=256] over 32
    k-tiles, lhsT = XT[mt] slice, rhs = WT chunk slice.
"""

import numpy as np

P = 128
M = 2048  # rows per core (one batch)
K = 4096  # in_features
N = 4096  # out_features
MT = M // P  # 16 m-tiles
KT = K // P  # 32 k-tiles
CH = 256  # out_features per chunk
NCH = N // CH  # 16 chunks
RSLICE = N // 8  # 512 rows of W reduced per core

_CACHE = {}


def _build_program(
    n_cores,
    reps=1,
    stage_bufs=3,
    wt_bufs=2,
    psum_bufs=7,
    out_bufs=4,
    no_matmul=False,
    no_wstream=False,
    no_xprep=False,
    no_store=False,
    no_sign=False,
    no_scale=False,
    mm_const_rhs=False,
    sign_as_copy=False,
    sign_on_dve=False,
):
    import concourse.bass as bass
    import concourse.mybir as mybir
    import concourse.tile as tile
    from concourse import bacc

    fp32 = mybir.dt.float32
    bf16 = mybir.dt.bfloat16

    nc = bacc.Bacc("TRN2", target_bir_lowering=False, debug=False, num_devices=n_cores)

    x_d = nc.dram_tensor("x", [M, K], fp32, kind="ExternalInput").ap()
    w_d = nc.dram_tensor("weight", [N, K], fp32, kind="ExternalInput").ap()
    ws_d = nc.dram_tensor("wslice", [RSLICE, K], fp32, kind="ExternalInput").ap()
    o_d = nc.dram_tensor("out", [M, N], fp32, kind="ExternalOutput").ap()

    with tile.TileContext(nc) as tc:
        cc_in, cc_in_free = tc.tile(
            [P, 1], fp32, space=bass.MemorySpace.DRAM, name="cc_in"
        )
        cc_out, cc_out_free = tc.tile(
            [P, 1],
            fp32,
            space=bass.MemorySpace.DRAM,
            addr_space="Shared",
            name="cc_out",
        )
        with (
            tc.tile_pool(name="consts", bufs=1) as consts,
            tc.tile_pool(name="stage", bufs=stage_bufs) as stage,
            tc.tile_pool(name="xt", bufs=16) as xt_pool,
            tc.tile_pool(name="wt", bufs=wt_bufs) as wt_pool,
            tc.tile_pool(name="outp", bufs=out_bufs) as outp,
            tc.tile_pool(name="psum", bufs=psum_bufs, space="PSUM") as psum,
            tc.tile_pool(name="psb", bufs=1, space="PSUM") as psb,
        ):
            ones = consts.tile([P, P], fp32, name="ones")
            nc.vector.memset(ones[:], 1.0)
            if mm_const_rhs:
                wt_const = consts.tile([P, 2, KT, P], bf16, name="wt_const")
                nc.vector.memset(wt_const[:], 1.0)
            racc = consts.tile([P, 4], fp32, name="racc")
            red1 = consts.tile([P, 1], fp32, name="red1")
            ccs = consts.tile([P, 1], fp32, name="ccs")
            scale_t = consts.tile([P, 1], fp32, name="scale_t")

            # ---- scale: |W| partial over this core's row slice, AllReduce ----
            if no_scale:
                nc.vector.memset(scale_t[:], 0.015)
            for rt in range(4 if not no_scale else 0):
                st = stage.tile([P, K], bf16, tag="stage", name="red_st")
                nc.gpsimd.dma_start(st[:], ws_d[rt * P : (rt + 1) * P, :])
                nc.vector.tensor_reduce(
                    racc[:, rt : rt + 1],
                    st[:],
                    axis=mybir.AxisListType.X,
                    op=mybir.AluOpType.add,
                    apply_absolute_value=True,
                )
            if not no_scale:
                nc.vector.tensor_reduce(
                    red1[:], racc[:], axis=mybir.AxisListType.X, op=mybir.AluOpType.add
                )
                nc.sync.dma_start(cc_in[:], red1[:])
                nc.gpsimd.collective_compute(
                    "AllReduce",
                    mybir.AluOpType.add,
                    replica_groups=[list(range(n_cores))],
                    ins=[cc_in[:]],
                    outs=[cc_out[:]],
                )
                nc.sync.dma_start(ccs[:], cc_out[:])
                ps1 = psb.tile([P, 1], fp32, name="ps1")
                nc.tensor.matmul(ps1[:], ones[:], ccs[:], start=True, stop=True)
                nc.scalar.mul(scale_t[:], ps1[:], 1.0 / (float(N) * float(K)))

            for _rep in range(reps):
                # ---- W chunks 0,1 first so chunk-0 prep overlaps x load ----
                wtcs = {}
                def prep_chunk(c):
                    # sign commutes with transpose: apply it on the 2D stage
                    # tile so the transposed wt chunk is written only by the
                    # xbar DMA (ACT in-place on the transposed tile measured
                    # pathologically slow on HW).
                    wtc = wt_pool.tile([P, 2, KT, P], bf16, tag="wt", name="wtc")
                    if not no_wstream:
                        for sub in range(2):
                            rt = 2 * c + sub
                            st = stage.tile([P, K], bf16, tag="stage", name="w_st")
                            nc.gpsimd.dma_start(st[:], w_d[rt * P : (rt + 1) * P, :])
                            if not no_sign:
                                if sign_as_copy:
                                    nc.scalar.copy(st[:], st[:])
                                elif sign_on_dve:
                                    nc.vector.tensor_scalar(
                                        out=st[:], in0=st[:],
                                        scalar1=0.0, scalar2=None,
                                        op0=mybir.AluOpType.is_ge,
                                    )
                                    nc.vector.tensor_scalar(
                                        out=st[:], in0=st[:],
                                        scalar1=2.0, scalar2=-1.0,
                                        op0=mybir.AluOpType.mult,
                                        op1=mybir.AluOpType.add,
                                    )
                                else:
                                    nc.scalar.sign(st[:], st[:])
                            nc.sync.dma_start(wtc[:, sub], st[:], transpose=True)
                    else:
                        nc.vector.memset(wtc[:], 1.0)
                    return wtc
                for c in range(2):
                    wtcs[c] = prep_chunk(c)

                # ---- x: cast-DMA to bf16, xbar-transpose into XT tiles ----
                xts = []
                for mt in range(MT):
                    xt = xt_pool.tile([P, KT, P], bf16, tag="xt", name="xt")
                    if not no_xprep:
                        st = stage.tile([P, K], bf16, tag="stage", name="x_st")
                        nc.gpsimd.dma_start(st[:], x_d[mt * P : (mt + 1) * P, :])
                        nc.sync.dma_start(xt[:], st[:], transpose=True)
                    else:
                        if mt == 0:
                            nc.vector.memset(xt[:], 0.5)
                    xts.append(xt)

                # ---- main chunk loop ----
                for c in range(NCH):
                    if c in wtcs:
                        wtc = wtcs[c]
                    else:
                        wtc = prep_chunk(c)
                    for mt in range(MT):
                        ps = psum.tile([P, CH], fp32, tag="ps", name="ps")
                        rhs = wt_const[:, :, :, :] if mm_const_rhs else wtc[:, :, :, :]
                        if not no_matmul:
                            for k in range(KT):
                                nc.tensor.matmul(
                                    ps[:],
                                    xts[mt][:, k, :],
                                    rhs[:, :, k, :],
                                    start=(k == 0),
                                    stop=(k == KT - 1),
                                )
                        else:
                            nc.vector.memset(ps[:], 0.0)
                        ob = outp.tile([P, CH], fp32, tag="ob", name="ob")
                        nc.scalar.activation(
                            ob[:],
                            ps[:],
                            mybir.ActivationFunctionType.Copy,
                            scale=scale_t[:],
                        )
                        if not no_store:
                            nc.sync.dma_start(
                                o_d[mt * P : (mt + 1) * P, c * CH : (c + 1) * CH], ob[:]
                            )

        cc_in_free()
        cc_out_free()

    nc.compile()
    return nc


def _get_runner(n_cores=8, reps=1):
    key = (n_cores, reps)
    if key not in _CACHE:
        nc = _build_program(n_cores, reps=reps)
        _CACHE[key] = _Runner(nc, n_cores)
    return _CACHE[key]


class _Runner:
    """Holds the compiled program and the jitted PJRT callable so repeat
    invocations skip retracing/recompiling."""

    def __init__(self, nc, n_cores):
        import jax
        import concourse.mybir as mybir
        import concourse.bass2jax as b2j

        self.n_cores = n_cores
        self.nc = nc
        captured = {}
        orig_jit = jax.jit

        def spy_jit(fn, **kw):
            jitted = orig_jit(fn, **kw)
            captured["fn"] = jitted
            return jitted

        self.in_names = []
        self.out_names = []
        self.out_shapes = {}
        in_specs = {}
        partition_name = nc.partition_id_tensor.name if nc.partition_id_tensor else None
        for alloc in nc.m.functions[0].allocations:
            if not isinstance(alloc, mybir.MemoryLocationSet):
                continue
            name = alloc.memorylocations[0].name
            if alloc.kind == "ExternalInput" and name != partition_name:
                self.in_names.append(name)
                in_specs[name] = (tuple(alloc.tensor_shape), mybir.dt.np(alloc.dtype))
            elif alloc.kind == "ExternalOutput":
                self.out_names.append(name)
                self.out_shapes[name] = (
                    tuple(alloc.tensor_shape),
                    mybir.dt.np(alloc.dtype),
                )

        b2j.jax.jit = spy_jit
        try:
            dummy = [
                {n: np.zeros(s, d) for n, (s, d) in in_specs.items()}
                for _ in range(n_cores)
            ]
            b2j.run_bass_via_pjrt(nc, dummy, n_cores=n_cores)
        finally:
            b2j.jax.jit = orig_jit
        assert "fn" in captured
        self.fn = captured["fn"]

    def run(self, in_maps):
        import jax

        args = []
        for name in self.in_names:
            args.append(np.concatenate([np.asarray(m[name]) for m in in_maps], axis=0))
        for name in self.out_names:
            shape, d = self.out_shapes[name]
            args.append(np.zeros((self.n_cores * shape[0], *shape[1:]), d))
        out = self.fn(*args)
        jax.block_until_ready(out)
        res = []
        for c in range(self.n_cores):
            d = {}
            for i, name in enumerate(self.out_names):
                shape, _ = self.out_shapes[name]
                d[name] = np.asarray(out[i]).reshape(self.n_cores, *shape)[c]
            res.append(d)
        return res


def kernel(x: np.ndarray, weight: np.ndarray) -> np.ndarray:
    assert x.shape == (8, M, K) and weight.shape == (N, K)
    x = np.ascontiguousarray(x, dtype=np.float32)
    weight = np.ascontiguousarray(weight, dtype=np.float32)
    runner = _get_runner(8)
    in_maps = [
        {
            "x": x[b],
            "weight": weight,
            "wslice": weight[b * RSLICE : (b + 1) * RSLICE, :],
        }
        for b in range(8)
    ]
    res = runner.run(in_maps)
    return np.stack([res[b]["out"] for b in range(8)], axis=0)

